# revision 21
# baseline (speedup 1.0000x reference)
"""Single-head causal self-attention on 8 NeuronCores (data-parallel over batch).

Reference computation (per batch element b):
    Q = X @ Wq + bq; K = X @ Wk + bk; V = X @ Wv + bv        # [T, DK]
    S = Q @ K.T / sqrt(DK)  (causal masked)
    out = softmax(S) @ V                                      # [T, DK]

Device strategy (one batch element per core), fp16 compute / fp32 accumulate:
  - Host passes X.T [C, T] in fp16: contiguous DMA rows, half the
    HBM/tunnel traffic of fp32, and fp16 stationaries load via LDWEIGHTS
    that overlap the previous matmul (fp32 self-loads serialize).
  - X tiles are DMA'd in 0.25MB halves round-robin across the three DMA
    queues, stationaries at the queue heads, so the PE starts ~13us in
    and k-tiles keep arriving just ahead of the PE.
  - Two full-width projection passes with packed fp16 stationaries:
      pass A: [Wv | Wk] -> psum rows 0:64 = V.T, rows 64:128 = K.T
      pass B: [Wq | Wq] -> Q.T duplicated in both partition halves
    accumulated in eight per-512-chunk psum tiles (all 8 banks); chunk
    order is interleaved A0,B0,A1,B1,... so early chunks' accumulation
    groups close first and their drains overlap the projection tail.
  - Drains (exact bias add, fp16 out) are split: V/K chunks on VectorE,
    Q chunks on GpSimd, so both run in parallel right after each chunk's
    group closes; V transposes then only wait for their own chunk.
  - V.T is PE-transposed into natural [s, dk] tiles with a ones column
    appended; the ones column makes the output matmul also produce the
    softmax denominator l (row 64 of the output).
  - Scores are computed transposed: S.T[s, t] = K.T^T @ Q.T, one matmul
    per 1024-block (non-accumulating matmuls may span both psum banks of
    the block's tile), trimmed to start exactly at the diagonal column
    ts = 128*i.  exp (ScalarE, f32 psum -> fp16, scale=1/8 fused) per
    block; triangular mask multiply only on the diagonal 128-block.
  - PV: po[j][:, max(512j,ts):] += [V_i|1]^T @ exp-piece for j=i//4..3
    (sub-diagonal columns skipped entirely - no memsets).  Tensor-queue
    order runs one s-tile of score lookahead (S(i+1) before PV(i)) so
    the PE does not stall waiting for exp(i).
  - Device output per core: [65, T] bf16 = rows 0:64 unnormalized O.T,
    row 64 the denominator l (fp16 would overflow: l reaches ~3e5).
    Each chunk's writeback is split across the three DMA queues.  Host
    computes (O_unnorm / l).T in fp32.
"""

import sys

sys.path.insert(0, "/opt/trn_rl_repo")

import numpy as np

B, T, C, DK = 8, 2048, 1024, 64
KT = C // 128          # 8 k-tiles in the contraction over C
NS = T // 128          # 16 s-tiles (key blocks)
NCHUNK = T // 512      # 4 output chunks of 512
SCALE = 1.0 / np.sqrt(DK)

_CACHE = {}


def _build():
    from concourse import bass, bacc, tile

    mybir = bass.mybir
    f32 = mybir.dt.float32
    f16 = mybir.dt.float16
    bf16 = mybir.dt.bfloat16

    nc = bacc.Bacc(
        "TRN2", target_bir_lowering=False, debug=False, num_devices=B
    )

    xt_d = nc.dram_tensor("xt", [KT, 128, T], f16, kind="ExternalInput")
    wvk_d = nc.dram_tensor("wvk", [128, KT * 128], f16, kind="ExternalInput")
    wqq_d = nc.dram_tensor("wqq", [128, KT * 128], f16, kind="ExternalInput")
    bvk_d = nc.dram_tensor("bvk", [128, 1], f32, kind="ExternalInput")
    bqq_d = nc.dram_tensor("bqq", [128, 1], f32, kind="ExternalInput")
    out_d = nc.dram_tensor("out", [65, T], bf16, kind="ExternalOutput")

    # one packed const block: cols 0:128 tri-mask, 128:192 ident (rows 0:64)
    cst_np = np.zeros((128, 192), dtype=np.float16)
    cst_np[:, 0:128] = np.triu(np.ones((128, 128), dtype=np.float16))
    cst_np[0:64, 128:192] = np.eye(64, dtype=np.float16)
    cst_d = nc.inline_tensor(cst_np, "cst")

    EXP = mybir.ActivationFunctionType.Exp

    with tile.TileContext(nc) as tc:
        with tc.tile_pool(name="persist", bufs=1) as ppool:

            wvk = ppool.tile([128, KT * 128], f16)
            wqq = ppool.tile([128, KT * 128], f16)
            cst = ppool.tile([128, 192], f16)
            bvk = ppool.tile([128, 1], f32)
            bqq = ppool.tile([128, 1], f32)
            tri = cst[:, 0:128]
            ident = cst[0:64, 128:192]

            xts = [
                ppool.tile([128, T], f16, tag=f"x{k}", name=f"x{k}")
                for k in range(KT)
            ]

            # Hand-balanced DMA schedule (column-halves keep 2KB rows).
            # x tiles arrive in k order just ahead of the PE's consumption;
            # consts are only needed at drain time so they ride at the back
            # of the scalar queue.
            def xh(k, h):
                sl = slice(1024 * h, 1024 * (h + 1))
                return (xts[k], xt_d, k, sl)

            sched = {
                nc.sync: [(wvk, wvk_d), xh(0, 0), xh(2, 0), xh(3, 1),
                          xh(5, 0), xh(6, 1)],
                nc.gpsimd: [(wqq, wqq_d), xh(0, 1), xh(2, 1), xh(4, 0),
                            xh(5, 1), xh(7, 0)],
                nc.scalar: [xh(1, 0), xh(1, 1), xh(3, 0), xh(4, 1),
                            xh(6, 0), xh(7, 1), (cst, cst_d), (bvk, bvk_d),
                            (bqq, bqq_d)],
            }
            for eng, items in sched.items():
                for it in items:
                    if len(it) == 2:
                        tl, dr = it
                        eng.dma_start(out=tl[:], in_=dr[:])
                    else:
                        tl, dr, k, sl = it
                        eng.dma_start(out=tl[:, sl], in_=dr[k, :, sl])

            # persistent activations; vk/qq are per-chunk tiles so each
            # consumer waits only on its own chunk's drain
            vks = [
                ppool.tile([128, 512], f16, tag=f"vk{c}", name=f"vk{c}")
                for c in range(NCHUNK)
            ]
            qqs = [
                ppool.tile([128, 1024], f16, tag=f"qq{tb}", name=f"qq{tb}")
                for tb in range(2)
            ]
            v1 = ppool.tile([128, NS * 65], f16, tag="v1")  # [V_i | 1] stationaries
            osb = ppool.tile([65, T], bf16, tag="osb")

            nc.gpsimd.memset(v1[:], 1.0)

            # ---------------- projections ----------------
            with tc.tile_pool(name="pproj", bufs=1, space="PSUM") as pproj:
                psA = [
                    pproj.tile([128, 512], f32, tag=f"psA{c}", name=f"psA{c}")
                    for c in range(NCHUNK)
                ]
                psB = [
                    pproj.tile([128, 512], f32, tag=f"psB{c}", name=f"psB{c}")
                    for c in range(NCHUNK)
                ]
                for k in range(KT):
                    for c in range(NCHUNK):
                        sl = slice(512 * c, 512 * (c + 1))
                        for ps, w in ((psA, wvk), (psB, wqq)):
                            nc.tensor.matmul(
                                ps[c][:],
                                w[:, 128 * k:128 * (k + 1)],
                                xts[k][:, sl],
                                start=(k == 0), stop=(k == KT - 1),
                            )
                # drains: V/K on VectorE, Q on ScalarE (parallel; GpSimd
                # cannot touch PSUM).  ScalarE's Identity activation with a
                # per-partition AP bias is an exact bias add.
                IDENT_FN = mybir.ActivationFunctionType.Identity
                for c in range(NCHUNK):
                    nc.vector.tensor_scalar_add(vks[c][:], psA[c][:], bvk[:])
                    nc.scalar.activation(
                        qqs[c // 2][:, 512 * (c % 2):512 * (c % 2) + 512],
                        psB[c][:], IDENT_FN, bias=bqq[:],
                    )

            # ------------- V transposes + attention (one scope) -------------
            # PSUM: po 4 banks + st 3x[128,512] + vt 1 bank = 8.
            # Emission interleaves transposes with early score tiles so the
            # PE flows straight from projections into attention; v1 copies
            # run on GpSimd (idle), drains on VectorE/ScalarE.
            with tc.tile_pool(name="po", bufs=1, space="PSUM") as po, \
                 tc.tile_pool(name="pst", bufs=1, space="PSUM") as pst, \
                 tc.tile_pool(name="et", bufs=3) as etpool:

                ops = [
                    po.tile([65, 512], f32, tag=f"o{j}", name=f"o{j}")
                    for j in range(NCHUNK)
                ]

                ets = {}

                def transpose4(c):
                    for i in range(4 * c, 4 * c + 4):
                        vt = pst.tile([128, 64], f16, tag="vt", name="vt")
                        nc.tensor.transpose(
                            vt[:],
                            vks[i // 4][0:64, 128 * (i % 4):128 * (i % 4) + 128],
                            ident[:],
                        )
                        nc.vector.tensor_copy(v1[:, 65 * i:65 * i + 64], vt[:])

                def scores_exp(i):
                    ts = 128 * i
                    et = etpool.tile([128, T], f16, tag="et", name="et")
                    ets[i] = et
                    t0 = ts
                    while t0 < T:
                        t1 = min(T, (t0 // 512 + 1) * 512)
                        st = pst.tile([128, 512], f32, tag="st", name="st",
                                      bufs=3)
                        nc.tensor.matmul(
                            st[:, 0:t1 - t0],
                            vks[i // 4][
                                64:128, 128 * (i % 4):128 * (i % 4) + 128
                            ],
                            qqs[t0 // 1024][64:128, t0 % 1024:t0 % 1024 + t1 - t0],
                            start=True, stop=True,
                        )
                        nc.scalar.activation(
                            et[:, t0:t1], st[:, 0:t1 - t0], EXP, scale=SCALE,
                        )
                        if t0 == ts:  # diagonal block is in this piece
                            nc.vector.tensor_mul(
                                et[:, ts:ts + 128], et[:, ts:ts + 128], tri[:]
                            )
                        t0 = t1

                def pv_acc(i):
                    ts = 128 * i
                    jmin = i // 4
                    et = ets.pop(i)
                    for j in range(jmin, NCHUNK):
                        c0 = max(512 * j, ts)
                        nc.tensor.matmul(
                            ops[j][:, c0 - 512 * j:],
                            v1[:, 65 * i:65 * i + 65],
                            et[:, c0:512 * (j + 1)],
                            start=(i == 0), stop=(i == 4 * j + 3),
                        )
                    for j in range(jmin, NCHUNK):
                        if i == 4 * j + 3:
                            sl = slice(512 * j, 512 * (j + 1))
                            nc.vector.tensor_copy(osb[:, sl], ops[j][:])
                            for h, eng in enumerate(
                                (nc.sync, nc.gpsimd, nc.scalar)
                            ):
                                c0 = 512 * j + 171 * h
                                c1 = 512 * j + (171 * (h + 1) if h < 2 else 512)
                                eng.dma_start(
                                    out=out_d[:, c0:c1], in_=osb[:, c0:c1]
                                )

                transpose4(0)
                scores_exp(0)
                transpose4(1)
                scores_exp(1)
                pv_acc(0)
                transpose4(2)
                scores_exp(2)
                pv_acc(1)
                transpose4(3)
                for i in range(2, NS):
                    if i + 1 < NS:
                        scores_exp(i + 1)
                    pv_acc(i)

    nc.compile()
    return nc


def _get_nc():
    if "nc" not in _CACHE:
        _CACHE["nc"] = _build()
    return _CACHE["nc"]


def make_in_maps(X, Wq, bq, Wk, bk, Wv, bv):
    X = np.asarray(X, dtype=np.float32)
    Wq = np.asarray(Wq, dtype=np.float32)
    Wk = np.asarray(Wk, dtype=np.float32)
    Wv = np.asarray(Wv, dtype=np.float32)
    bq = np.asarray(bq, dtype=np.float32)
    bk = np.asarray(bk, dtype=np.float32)
    bv = np.asarray(bv, dtype=np.float32)

    wvk = np.ascontiguousarray(
        np.concatenate([Wv, Wk], axis=1).reshape(KT, 128, 128)
        .transpose(1, 0, 2).reshape(128, KT * 128)
    ).astype(np.float16)
    wqq = np.ascontiguousarray(
        np.concatenate([Wq, Wq], axis=1).reshape(KT, 128, 128)
        .transpose(1, 0, 2).reshape(128, KT * 128)
    ).astype(np.float16)
    bvk = np.concatenate([bv, bk]).reshape(128, 1).astype(np.float32)
    bqq = np.concatenate([bq, bq]).reshape(128, 1).astype(np.float32)

    in_maps = []
    for b in range(B):
        xt = (
            np.ascontiguousarray(X[b].T)
            .astype(np.float16)
            .reshape(KT, 128, T)
        )
        in_maps.append(
            {"xt": xt, "wvk": wvk, "wqq": wqq, "bvk": bvk, "bqq": bqq}
        )
    return in_maps


def kernel(X, Wq, bq, Wk, bk, Wv, bv):
    from concourse.bass_utils import run_bass_kernel_spmd

    nc = _get_nc()
    in_maps = make_in_maps(X, Wq, bq, Wk, bk, Wv, bv)
    res = run_bass_kernel_spmd(nc, in_maps, list(range(B)))

    out = np.empty((B, T, DK), dtype=np.float32)
    for b in range(B):
        r = np.asarray(res.results[b]["out"], dtype=np.float32)
        out[b] = (r[:64] / r[64:65]).T
    return out
